# revision 6
# baseline (speedup 1.0000x reference)
"""Trainium2 Bass kernel for DynamicFilterWithImageInput.

Model (per batch b):
  img_feat = mean_hw(relu(BN1(conv2d(raw_img, w_conv1, 3x3, zeropad=1) + b1)))   # (64,)
  df       = softmax_over_C(BN2(img_feat @ w_filt.T + b_filt).reshape(C, K*K))   # (C, 25)
  out      = depthwise_conv5x5(reflect_pad(x_feat), df)                          # (C, H, W)

Sharding: pure data-parallel over batch (16 batches -> 8 cores x 2 batches).

Device mapping (per core, B_PC=2 batches):
  - conv1 as a single K=54 matmul (host-built im2col + block-diag weights,
    BN1 folded in), ReLU+bias+spatial-sum on ScalarE (accum_out), mean+dense
    (K=65 with bias row folded in), softmax on [50, 256] layout
    (partition=(b,tap), free=channel), all tiny.
  - depthwise 5x5: for each slab (b, channel-group-of-128): 25 taps as
    diag-weight matmuls accumulating in PSUM; a few taps offloaded to
    VectorE via fused scalar_tensor_tensor (psum += x * w[c]).  Moving data
    fp16, diag weights fp16, PSUM accumulation fp32.  Output DMA'd straight
    from PSUM to HBM.
"""

import os
import sys

sys.path.insert(0, "/opt/trn_rl_repo")

import numpy as np

import concourse.bass as bass
import concourse.bacc as bacc
import concourse.mybir as mybir
import concourse.tile as tile
from concourse.bass_utils import run_bass_kernel_spmd

F16 = mybir.dt.float16
F32 = mybir.dt.float32
AF = mybir.ActivationFunctionType
ALU = mybir.AluOpType

EPS = 1e-5
N_CORES = 8
B_PC = 2          # batches per core
C = 256           # channels
CG = C // 128     # channel groups of 128
K5 = 5            # depthwise kernel size
TAPS = [(i, j) for i in range(K5) for j in range(K5)]

_PROG_CACHE = {}


def _build_program(H, W, n_dve_taps):
    """Emit the per-core Tile program. Returns compiled nc."""
    Hp, Wp = H + 4, W + 4
    HWOUT = H * W
    RCHUNK = min(H, max(1, 512 // W))    # output rows per psum chunk
    assert H % RCHUNK == 0
    NCHUNK = H // RCHUNK
    N1CH = min(512, HWOUT)               # conv1 psum chunk
    assert HWOUT % N1CH == 0
    N1 = HWOUT // N1CH                   # number of conv1 chunks

    assert n_dve_taps >= 1  # last DVE tap evacuates PSUM -> SBUF
    pe_taps = TAPS[: 25 - n_dve_taps]
    dve_taps = TAPS[25 - n_dve_taps:]

    nc = bacc.Bacc("TRN2", target_bir_lowering=False, debug=False)

    x_d = nc.dram_tensor("x", [B_PC, C, H, W], F16, kind="ExternalInput").ap()
    im2col_d = nc.dram_tensor("im2col", [54, HWOUT], F16, kind="ExternalInput").ap()
    wconv_d = nc.dram_tensor("wconv", [54, 128], F16, kind="ExternalInput").ap()
    b1r_d = nc.dram_tensor("b1r", [128, 1], F32, kind="ExternalInput").ap()
    wft_d = nc.dram_tensor("wft", [65, C, 25], F16, kind="ExternalInput").ap()
    ident_d = nc.dram_tensor("ident", [128, 128], F16, kind="ExternalInput").ap()
    out_d = nc.dram_tensor("out", [B_PC, C, H, W], F32, kind="ExternalOutput").ap()

    # dram scratch for layout bounces
    imgf_d = nc.dram_tensor("imgf_sc", [128], F32).ap()
    df_d = nc.dram_tensor("df_sc", [B_PC, 25, C], F32).ap()
    wsm_d = nc.dram_tensor("wsm_sc", [B_PC, 25, C], F32).ap()

    with tile.TileContext(nc) as tc:
        with (
            tc.tile_pool(name="consts", bufs=1) as consts,
            tc.tile_pool(name="p0", bufs=1) as p0,
            tc.tile_pool(name="trash", bufs=2) as trashp,
            tc.tile_pool(name="xp", bufs=2) as xpp,
            tc.tile_pool(name="diag", bufs=2) as diagp,
            tc.tile_pool(name="p0psum", bufs=2, space="PSUM") as p0psum,
            tc.tile_pool(name="dpsum", bufs=2, space="PSUM") as dpsum,
            tc.tile_pool(name="dwpsum", bufs=4, space="PSUM") as dwpsum,
        ):
            # ---------- phase 0: filter generation ----------
            im2col = consts.tile([54, HWOUT], F16)
            wconv = consts.tile([54, 128], F16)
            b1r = consts.tile([128, 1], F32)
            wft = consts.tile([65, C, 25], F16)
            ident = consts.tile([128, 128], F16)
            nc.sync.dma_start(im2col[:], im2col_d[:])
            nc.sync.dma_start(wconv[:], wconv_d[:])
            nc.sync.dma_start(b1r[:], b1r_d[:])
            nc.sync.dma_start(wft[:], wft_d[:])
            nc.sync.dma_start(ident[:], ident_d[:])

            acc = p0.tile([128, N1], F32)
            for ci in range(N1):
                ps1 = p0psum.tile([128, N1CH], F32)
                nc.tensor.matmul(
                    ps1[:], wconv[:], im2col[:, ci * N1CH:(ci + 1) * N1CH],
                    start=True, stop=True,
                )
                tr = trashp.tile([128, N1CH], F32)
                nc.scalar.activation(
                    tr[:], ps1[:], AF.Relu, bias=b1r[:], scale=1.0,
                    accum_out=acc[:, ci:ci + 1],
                )
            sfeat = p0.tile([128, 1], F32)
            if N1 > 1:
                nc.vector.tensor_reduce(sfeat[:], acc[:], mybir.AxisListType.X, ALU.add)
            else:
                nc.vector.tensor_copy(sfeat[:], acc[:])
            mfeat = p0.tile([128, 1], F32)
            nc.scalar.mul(mfeat[:], sfeat[:], 1.0 / HWOUT)
            nc.sync.dma_start(imgf_d[:], mfeat[:])

            # img_feat transposed [64,2] + ones row -> [65,2] fp16
            imgfT32 = p0.tile([65, B_PC], F32)
            nc.sync.dma_start(
                imgfT32[0:64, :],
                imgf_d[:].rearrange("(b o) -> o b", b=B_PC, o=64),
            )
            imgfT = p0.tile([65, B_PC], F16)
            nc.vector.tensor_copy(imgfT[0:64, :], imgfT32[0:64, :])
            nc.vector.memset(imgfT[64:65, :], 1.0)

            # dense: df[b, t, c] (+bias row), chunks of 2 taps
            for t0 in range(0, 25, 2):
                tw = min(2, 25 - t0)
                psd = dpsum.tile([B_PC, tw, C], F32)
                nc.tensor.matmul(
                    psd[:], imgfT[:],
                    wft[:, :, t0:t0 + tw].transpose([0, 2, 1]),
                    start=True, stop=True,
                )
                dfc = trashp.tile([B_PC, tw, C], F32, tag="dfc")
                nc.scalar.copy(dfc[:], psd[:])
                nc.sync.dma_start(df_d[:, t0:t0 + tw, :], dfc[:])

            # softmax over channels on [50, 256]
            dfsb = p0.tile([B_PC * 25, C], F32)
            nc.sync.dma_start(dfsb[:], df_d[:].flatten_outer_dims())
            edf = p0.tile([B_PC * 25, C], F32)
            nc.scalar.activation(edf[:], dfsb[:], AF.Exp)
            ssum = p0.tile([B_PC * 25, 1], F32)
            nc.vector.tensor_reduce(ssum[:], edf[:], mybir.AxisListType.X, ALU.add)
            rsum = p0.tile([B_PC * 25, 1], F32)
            nc.vector.reciprocal(rsum[:], ssum[:])
            wsm = p0.tile([B_PC * 25, C], F32)
            nc.vector.tensor_scalar(wsm[:], edf[:], rsum[:], None, ALU.mult)
            nc.sync.dma_start(wsm_d[:].flatten_outer_dims(), wsm[:])

            # per-slab filter values [128(c), 25] fp32
            vts = []
            for s in range(B_PC * CG):
                b, cg = divmod(s, CG)
                vt = p0.tile([128, 25], F32, tag="vt")
                nc.sync.dma_start(
                    vt[:], wsm_d[b, :, cg * 128:(cg + 1) * 128].transpose([1, 0])
                )
                vts.append(vt)

            # ---------- depthwise ----------
            for s in range(B_PC * CG):
                b, cg = divmod(s, CG)
                vt = vts[s]

                dtile = diagp.tile([128, 25, 128], F16)
                for t in range(25):
                    nc.scalar.mul(dtile[:, t, :], ident[:], vt[:, t:t + 1])

                xp = xpp.tile([128, Hp, Wp], F16)
                nc.sync.dma_start(
                    xp[:, 2:2 + H, 2:2 + W], x_d[b, cg * 128:(cg + 1) * 128, :, :]
                )
                # reflect pads: rows then cols (corners via cols pass)
                for dst, src in ((1, 3), (0, 4), (Hp - 2, Hp - 4), (Hp - 1, Hp - 5)):
                    nc.scalar.copy(xp[:, dst, 2:2 + W], xp[:, src, 2:2 + W])
                for dst, src in ((1, 3), (0, 4), (Wp - 2, Wp - 4), (Wp - 1, Wp - 5)):
                    nc.scalar.copy(xp[:, :, dst], xp[:, :, src])

                for ychunk in range(NCHUNK):
                    y0 = ychunk * RCHUNK
                    ps = dwpsum.tile([128, RCHUNK, W], F32)
                    for k, (i, j) in enumerate(pe_taps):
                        nc.tensor.matmul(
                            ps[:],
                            dtile[:, i * K5 + j, :],
                            xp[:, y0 + i:y0 + i + RCHUNK, j:j + W],
                            start=(k == 0),
                            stop=(k == len(pe_taps) - 1),
                        )
                    ot = trashp.tile([128, RCHUNK, W], F32, tag="ot")
                    for k, (i, j) in enumerate(dve_taps):
                        last = k == len(dve_taps) - 1
                        nc.vector.scalar_tensor_tensor(
                            ot[:] if last else ps[:],
                            xp[:, y0 + i:y0 + i + RCHUNK, j:j + W],
                            vt[:, i * K5 + j:i * K5 + j + 1],
                            ps[:],
                            ALU.mult,
                            ALU.add,
                        )
                    nc.sync.dma_start(
                        out_d[b, cg * 128:(cg + 1) * 128, y0:y0 + RCHUNK, :], ot[:]
                    )

    nc.compile()
    return nc


def get_program(H, W, n_dve_taps=6):
    key = (H, W, n_dve_taps)
    if key not in _PROG_CACHE:
        _PROG_CACHE[key] = _build_program(H, W, n_dve_taps)
    return _PROG_CACHE[key]


def host_prep(x_feat, raw_img, w_conv1, b_conv1, g1, beta1, m1, v1,
              w_filt, b_filt, g2, beta2, m2, v2):
    """Fold BN params, build im2col + packed weights; returns per-core in_maps."""
    B, Cc, H, W = x_feat.shape
    assert Cc == C
    n_cores = B // B_PC

    a1 = g1 / np.sqrt(v1 + EPS)
    w1f = (w_conv1 * a1[:, None, None, None]).astype(np.float32)   # (64,3,3,3)
    b1f = (b_conv1 - m1) * a1 + beta1                               # (64,)

    a2 = g2 / np.sqrt(v2 + EPS)
    wff = (w_filt * a2[:, None]).astype(np.float32)                 # (6400,64)
    bff = (b_filt - m2) * a2 + beta2                                # (6400,)

    # wft[k, c, t]: k<64 -> wff[c*25+t, k]; k=64 -> bias row
    wft = np.empty((65, C, 25), np.float32)
    wft[:64] = wff.reshape(C, 25, 64).transpose(2, 0, 1)
    wft[64] = bff.reshape(C, 25)
    wft16 = wft.astype(np.float16)

    b1r = np.tile(b1f, B_PC).reshape(128, 1).astype(np.float32)

    ident = np.eye(128, dtype=np.float16)

    x16 = x_feat.astype(np.float16)

    # conv1 im2col, zero pad 1: [54, H*W] per core
    rawpad = np.pad(raw_img, ((0, 0), (0, 0), (1, 1), (1, 1))).astype(np.float32)

    # wconv[b*27 + (c*9+i*3+j), b*64+o] = w1f[o, c, i, j]
    wconv = np.zeros((54, 128), np.float32)
    w_flat = w1f.transpose(1, 2, 3, 0).reshape(27, 64)  # (c*9+i*3+j, o)
    for b in range(B_PC):
        wconv[b * 27:(b + 1) * 27, b * 64:(b + 1) * 64] = w_flat
    wconv16 = wconv.astype(np.float16)

    in_maps = []
    for core in range(n_cores):
        bs = core * B_PC
        im2col = np.empty((54, H * W), np.float32)
        for b in range(B_PC):
            for c in range(3):
                for i in range(3):
                    for j in range(3):
                        p = b * 27 + c * 9 + i * 3 + j
                        im2col[p] = rawpad[bs + b, c, i:i + H, j:j + W].reshape(-1)
        in_maps.append({
            "x": x16[bs:bs + B_PC],
            "im2col": im2col.astype(np.float16),
            "wconv": wconv16,
            "b1r": b1r,
            "wft": wft16,
            "ident": ident,
        })
    return in_maps


def run(inputs, trace=False, n_dve_taps=6):
    x_feat = inputs["x_feat"]
    B, _, H, W = x_feat.shape
    nc = get_program(H, W, n_dve_taps)
    in_maps = host_prep(**inputs)
    n_cores = len(in_maps)
    res = run_bass_kernel_spmd(nc, in_maps, list(range(n_cores)), trace=trace)
    out = np.concatenate([r["out"] for r in res.results], axis=0)
    return out, res


def kernel(**inputs) -> np.ndarray:
    out, _ = run(inputs, trace=False)
    return out
